# revision 19
# baseline (speedup 1.0000x reference)
"""Multi-head causal attention (B=2, S=2048, H=16, Dh=64) on 8 TRN2 NeuronCores.

Sharding: tensor-parallel over heads — core c owns heads [2c, 2c+1] (a
128-wide feature block) for both batches. Each core computes its heads'
QKV projections, causal attention, and a partial output projection
(attn_c @ Wo[:, 128c:128c+128].T); the host sums the 8 partials
(the all-reduce) and reshapes.

On-chip layout keeps the feature dim on SBUF partitions throughout
("T layout"), so scores are computed directly as S.T[j, i] and the
post-softmax matrix needs no transpose before the PV matmul. The softmax
row-sum is obtained for free by augmenting V with 64 columns of ones
inside the PV matmul; exp is unnormalized (scores are O(1), no max
subtraction needed) and the division happens once per output tile.
"""

import numpy as np

import concourse.bass as bass
import concourse.mybir as mybir
import concourse.tile as tile
from concourse import bacc
from concourse.bass import ds
from concourse.masks import make_identity

B, S, H, Dh = 2, 2048, 16, 64
D = H * Dh            # 1024
NCORES = 8
HPC = H // NCORES     # heads per core = 2
M = HPC * Dh          # per-core feature block = 128
N = B * S             # 4096 token rows
IC = 512              # i-chunk (matmul moving free dim)
NICB = S // IC        # 4 i-chunks per batch
NJT = S // 128        # 16 j-tiles per batch

F32 = mybir.dt.float32
F32R = mybir.dt.float32r
BF16 = mybir.dt.bfloat16

# When True, the x activations and QKV projection weights are shipped and
# multiplied in bf16 (halves the dominant DMA stream). Scores, P, V, and the
# output projection stay fp32r.
USE_BF16_X = False
XDT = BF16 if USE_BF16_X else F32R
AF = mybir.ActivationFunctionType
ALU = mybir.AluOpType

MASK_NEG = -1.0e30


def _build_bass(bench_iters=None):
    nc = bacc.Bacc("TRN2", target_bir_lowering=False, debug=False,
                   num_devices=NCORES)

    xqT = nc.dram_tensor("xqT", [D, N], XDT, kind="ExternalInput").ap()
    xkT = nc.dram_tensor("xkT", [D, N], XDT, kind="ExternalInput").ap()
    wqT = nc.dram_tensor("wqT", [D, M], XDT, kind="ExternalInput").ap()
    wkT = nc.dram_tensor("wkT", [D, M], XDT, kind="ExternalInput").ap()
    wvT = nc.dram_tensor("wvT", [D, M], XDT, kind="ExternalInput").ap()
    woT = nc.dram_tensor("woT", [M, D], F32R, kind="ExternalInput").ap()
    msk = nc.dram_tensor("msk", [128, 4 * IC], F32, kind="ExternalInput").ap()
    out = nc.dram_tensor("out", [N, D], F32, kind="ExternalOutput").ap()

    with tile.TileContext(nc) as tc:
        with (
            tc.tile_pool(name="wts", bufs=1) as wpool,
            tc.tile_pool(name="xs", bufs=3) as xpool,
            tc.tile_pool(name="acts", bufs=2) as apool,
            tc.tile_pool(name="ps", bufs=4) as ppool,
            tc.tile_pool(name="qkv", bufs=2, space="PSUM") as qkvps,
            tc.tile_pool(name="sc", bufs=2, space="PSUM") as scps,
            tc.tile_pool(name="pv", bufs=2, space="PSUM") as pvps,
            tc.tile_pool(name="op", bufs=2, space="PSUM") as opps,
        ):
            # --- constants ---
            wq_sb = wpool.tile([128, 8 * 128], XDT, tag="wq")
            wk_sb = wpool.tile([128, 8 * 128], XDT, tag="wk")
            wv_sb = wpool.tile([128, 8 * 128], XDT, tag="wv")
            wo_sb = wpool.tile([128, D], F32R, tag="wo")
            msk_sb = wpool.tile([128, 4 * IC], F32, tag="msk")
            idn = wpool.tile([128, 128], F32, tag="idn")
            nc.sync.dma_start(wq_sb.rearrange("p (c m) -> p c m", m=128),
                              wqT.rearrange("(c p) m -> p c m", p=128))
            nc.sync.dma_start(wk_sb.rearrange("p (c m) -> p c m", m=128),
                              wkT.rearrange("(c p) m -> p c m", p=128))
            nc.sync.dma_start(wv_sb.rearrange("p (c m) -> p c m", m=128),
                              wvT.rearrange("(c p) m -> p c m", p=128))
            nc.sync.dma_start(wo_sb[:], woT[:, :])
            nc.sync.dma_start(msk_sb[:], msk[:, :])
            make_identity(nc, idn[:])

            from contextlib import nullcontext
            loop_cm = (tc.For_i(0, bench_iters, 1)
                       if bench_iters else nullcontext())
            with loop_cm:
                _emit_body(nc, tc, locals())
    nc.finalize()
    return nc


def _emit_body(nc, tc, env):
    (xqT, xkT, msk_sb, out, wq_sb, wk_sb, wv_sb, wo_sb, idn,
     xpool, apool, ppool, qkvps, scps, pvps, opps) = (
        env["xqT"], env["xkT"], env["msk_sb"], env["out"], env["wq_sb"],
        env["wk_sb"], env["wv_sb"], env["wo_sb"], env["idn"], env["xpool"],
        env["apool"], env["ppool"], env["qkvps"], env["scps"], env["pvps"],
        env["opps"])
    if True:
            for b in range(B):
                # per-batch activation tiles (bufs=2 double-buffers batches)
                qT = apool.tile([128, S], F32R, tag="qT")
                kT = apool.tile([128, S], F32R, tag="kT")
                # v_comb: per (j-tile, head) slot [128, 128]: cols 0-63 v,
                # cols 64-127 ones (for the fused row-sum)
                vc = apool.tile([128, NJT * HPC * 128], F32R, tag="vc")
                aT = apool.tile([128, S], F32R, tag="aT")

                ones_ap = vc.bitcast(F32).rearrange(
                    "p (s two c) -> p s two c", two=2, c=64)[:, :, 1, :]
                nc.gpsimd.memset(ones_ap, 1.0)

                # Interleaved per-i-chunk pipeline: QKV(icb) -> causal
                # attention(icb) -> partial out-projection(icb). Later
                # i-chunks' QKV DMA overlaps earlier chunks' attention.
                for icb in range(NICB):
                    i0 = b * S + icb * IC
                    # ---- QKV projections for this i-chunk ----
                    # One big DMA per source: all 8 d-chunks [128, 512]
                    xq_t = xpool.tile([128, 8, IC], XDT, tag="xa")
                    xk_t = xpool.tile([128, 8, IC], XDT, tag="xa")
                    for g in range(4):
                        nc.sync.dma_start(
                            xq_t[:, ds(2 * g, 2), :],
                            xqT[ds(2 * g * 128, 256), ds(i0, IC)].rearrange(
                                "(c p) i -> p c i", p=128))
                        nc.sync.dma_start(
                            xk_t[:, ds(2 * g, 2), :],
                            xkT[ds(2 * g * 128, 256), ds(i0, IC)].rearrange(
                                "(c p) i -> p c i", p=128))
                    for which, w_sb, x_t in (("q", wq_sb, xq_t),
                                             ("k", wk_sb, xk_t),
                                             ("v", wv_sb, xk_t)):
                        ps = qkvps.tile([128, IC], F32, tag="qkv",
                                        name=f"ps_{b}_{icb}_{which}")
                        for dc in range(8):
                            nc.tensor.matmul(ps[:], w_sb[:, ds(dc * 128, 128)],
                                             x_t[:, dc, :],
                                             start=(dc == 0), stop=(dc == 7))
                        if which == "q":
                            nc.vector.tensor_copy(qT[:, ds(icb * IC, IC)], ps[:])
                        elif which == "k":
                            nc.vector.tensor_copy(kT[:, ds(icb * IC, IC)], ps[:])
                        else:
                            # v -> natural [j, m] layout via PE transpose
                            vt_t = xpool.tile([128, IC], F32, tag="vt")
                            nc.vector.tensor_copy(vt_t[:], ps[:])
                            tp_ps = opps.tile([128, IC], F32, tag="op")
                            for t in range(4):
                                nc.tensor.transpose(tp_ps[:, ds(t * 128, 128)],
                                                    vt_t[:, ds(t * 128, 128)],
                                                    idn[:])
                            for t in range(4):
                                jt = icb * 4 + t
                                for h in range(HPC):
                                    nc.vector.tensor_copy(
                                        vc[:, ds((jt * HPC + h) * 128, 64)],
                                        tp_ps[:, ds(t * 128 + h * 64, 64)])

                    # ---- causal attention for this i-chunk ----
                    njt = 4 * icb + 4
                    pv_tiles = [pvps.tile([128, IC], F32, tag="pv",
                                          name=f"pv_{b}_{icb}_{h}")
                                for h in range(HPC)]
                    for jt in range(njt):
                        # both heads' K=64 score matmuls back-to-back: they
                        # target disjoint PE row-groups (partitions 0-63 /
                        # 64-127) and run concurrently in the array
                        s_tiles, p_ts = [], []
                        for h in range(HPC):
                            s_ps = scps.tile([128, IC], F32, tag="sc",
                                             name=f"s_{b}_{icb}_{jt}_{h}")
                            nc.tensor.matmul(
                                s_ps[:],
                                kT[ds(h * 64, 64), ds(jt * 128, 128)],
                                qT[ds(h * 64, 64), ds(icb * IC, IC)],
                                start=True, stop=True)
                            s_tiles.append(s_ps)
                        for h in range(HPC):
                            p_t = ppool.tile([128, IC], F32R, tag="p",
                                             name=f"p_{b}_{icb}_{jt}_{h}")
                            nc.scalar.activation(p_t[:], s_tiles[h][:], AF.Exp)
                            if jt >= 4 * icb:  # diagonal block: causal mask
                                rr = jt - 4 * icb
                                nc.vector.tensor_tensor(
                                    p_t[:], p_t[:],
                                    msk_sb[:, ds(rr * IC, IC)], ALU.mult)
                            p_ts.append(p_t)
                        for h in range(HPC):
                            nc.tensor.matmul(
                                pv_tiles[h][:],
                                vc[:, ds((jt * HPC + h) * 128, 128)],
                                p_ts[h][:],
                                start=(jt == 0), stop=(jt == njt - 1))
                    for h in range(HPC):
                        rc_t = ppool.tile([64, IC], F32, tag="rc")
                        nc.vector.reciprocal(rc_t[:], pv_tiles[h][ds(64, 64), :])
                        nc.vector.tensor_tensor(
                            aT[ds(h * 64, 64), ds(icb * IC, IC)],
                            pv_tiles[h][ds(0, 64), :],
                            rc_t[:], ALU.mult)

                    # ---- partial out-projection for this i-chunk ----
                    for i128 in range(4):
                        ii = icb * 4 + i128
                        for dn in range(D // IC):
                            o_ps = opps.tile([128, IC], F32, tag="op")
                            nc.tensor.matmul(o_ps[:],
                                             aT[:, ds(ii * 128, 128)],
                                             wo_sb[:, ds(dn * IC, IC)],
                                             start=True, stop=True)
                            o_sb = ppool.tile([128, IC], F32, tag="osb")
                            if dn == 0:
                                nc.vector.tensor_copy(o_sb[:], o_ps[:])
                            else:
                                nc.scalar.copy(o_sb[:], o_ps[:])
                            nc.sync.dma_start(
                                out[ds(b * S + ii * 128, 128), ds(dn * IC, IC)],
                                o_sb[:])


_STATE = {}


def _get_runner(bench_iters=None):
    """Build the Bass module and a cached jitted SPMD executor (compile once)."""
    global _STATE
    if bench_iters in _STATE:
        return _STATE[bench_iters]

    import jax
    from jax.sharding import Mesh, PartitionSpec
    from jax.experimental.shard_map import shard_map
    from concourse import bass2jax

    bass2jax.install_neuronx_cc_hook()
    nc = _build_bass(bench_iters)

    partition_name = (nc.partition_id_tensor.name
                      if nc.partition_id_tensor else None)
    in_names, out_names, out_avals, zero_shapes = [], [], [], []
    for alloc in nc.m.functions[0].allocations:
        if not isinstance(alloc, mybir.MemoryLocationSet):
            continue
        name = alloc.memorylocations[0].name
        if alloc.kind == "ExternalInput":
            if name != partition_name:
                in_names.append(name)
        elif alloc.kind == "ExternalOutput":
            shape = tuple(alloc.tensor_shape)
            dtype = mybir.dt.np(alloc.dtype)
            out_names.append(name)
            out_avals.append(jax.core.ShapedArray(shape, dtype))
            zero_shapes.append((shape, dtype))
    n_params = len(in_names)
    n_outs = len(out_avals)
    all_in_names = list(in_names) + list(out_names)
    if partition_name is not None:
        all_in_names.append(partition_name)

    def _body(*args):
        operands = list(args)
        if partition_name is not None:
            operands.append(bass2jax.partition_id_tensor())
        outs = bass2jax._bass_exec_p.bind(
            *operands,
            out_avals=tuple(out_avals),
            in_names=tuple(all_in_names),
            out_names=tuple(out_names),
            lowering_input_output_aliases=(),
            sim_require_finite=True,
            sim_require_nnan=True,
            nc=nc,
        )
        return tuple(outs)

    devices = jax.devices()[:NCORES]
    mesh = Mesh(np.asarray(devices), ("core",))
    in_specs = (PartitionSpec("core"),) * (n_params + n_outs)
    out_specs = (PartitionSpec("core"),) * n_outs
    donate = tuple(range(n_params, n_params + n_outs))
    sharded = jax.jit(
        shard_map(_body, mesh=mesh, in_specs=in_specs, out_specs=out_specs,
                  check_rep=False),
        donate_argnums=donate, keep_unused=True)

    def run(in_maps):
        concat_in = [
            np.concatenate([np.asarray(in_maps[c][k]) for c in range(NCORES)],
                           axis=0)
            for k in in_names
        ]
        concat_zeros = [np.zeros((NCORES * s[0], *s[1:]), dt)
                        for s, dt in zero_shapes]
        out_arrs = sharded(*concat_in, *concat_zeros)
        return [
            {k: np.asarray(out_arrs[i]).reshape(NCORES, *out_avals[i].shape)[c]
             for i, k in enumerate(out_names)}
            for c in range(NCORES)
        ]

    _STATE[bench_iters] = run
    return run


def _make_mask():
    """msk[jj, rr*512 + ii] = 1 if ii >= jj + 128*rr else 0 (multiplicative)."""
    jj = np.arange(128)[:, None]
    ii = np.arange(IC)[None, :]
    tiles = [np.where(ii >= jj + 128 * rr, 1.0, 0.0).astype(np.float32)
             for rr in range(4)]
    return np.concatenate(tiles, axis=1)


def prepare_in_maps(inputs_q, inputs_kv, Wq, Wk, Wv, Wo):
    if USE_BF16_X:
        import ml_dtypes
        xdt = ml_dtypes.bfloat16
    else:
        xdt = np.float32
    xq = np.ascontiguousarray(
        np.asarray(inputs_q, np.float32).reshape(N, D).T.astype(xdt))
    xk = np.ascontiguousarray(
        np.asarray(inputs_kv, np.float32).reshape(N, D).T.astype(xdt))
    Wq = np.asarray(Wq, np.float32)
    Wk = np.asarray(Wk, np.float32)
    Wv = np.asarray(Wv, np.float32)
    Wo = np.asarray(Wo, np.float32)
    msk = _make_mask()
    scale = 1.0 / np.sqrt(np.float32(Dh))
    in_maps = []
    for c in range(NCORES):
        sl = slice(c * M, (c + 1) * M)
        in_maps.append({
            "xqT": xq,
            "xkT": xk,
            "wqT": np.ascontiguousarray((Wq[sl, :] * scale).T.astype(xdt)),
            "wkT": np.ascontiguousarray(Wk[sl, :].T.astype(xdt)),
            "wvT": np.ascontiguousarray(Wv[sl, :].T.astype(xdt)),
            "woT": np.ascontiguousarray(Wo[:, sl].T),
            "msk": msk,
        })
    return in_maps


def kernel(inputs_q, inputs_kv, mask, Wq, Wk, Wv, Wo):
    run = _get_runner()
    in_maps = prepare_in_maps(inputs_q, inputs_kv, Wq, Wk, Wv, Wo)
    results = run(in_maps)
    acc = results[0]["out"].astype(np.float32)
    for c in range(1, NCORES):
        acc = acc + results[c]["out"]
    return acc.reshape(B, S, D)


# revision 20
# speedup vs baseline: 1.2791x; 1.2791x over previous
"""Multi-head causal attention (B=2, S=2048, H=16, Dh=64) on 8 TRN2 NeuronCores.

Sharding: tensor-parallel over heads — core c owns heads [2c, 2c+1] (a
128-wide feature block) for both batches. Each core computes its heads'
QKV projections, causal attention, and a partial output projection
(attn_c @ Wo[:, 128c:128c+128].T); the host sums the 8 partials
(the all-reduce) and reshapes.

On-chip layout keeps the feature dim on SBUF partitions throughout
("T layout"), so scores are computed directly as S.T[j, i] and the
post-softmax matrix needs no transpose before the PV matmul. The softmax
row-sum is obtained for free by augmenting V with 64 columns of ones
inside the PV matmul; exp is unnormalized (scores are O(1), no max
subtraction needed) and the division happens once per output tile.
"""

import numpy as np

import concourse.bass as bass
import concourse.mybir as mybir
import concourse.tile as tile
from concourse import bacc
from concourse.bass import ds
from concourse.masks import make_identity

B, S, H, Dh = 2, 2048, 16, 64
D = H * Dh            # 1024
NCORES = 8
HPC = H // NCORES     # heads per core = 2
M = HPC * Dh          # per-core feature block = 128
N = B * S             # 4096 token rows
IC = 512              # i-chunk (matmul moving free dim)
NICB = S // IC        # 4 i-chunks per batch
NJT = S // 128        # 16 j-tiles per batch

F32 = mybir.dt.float32
F32R = mybir.dt.float32r
BF16 = mybir.dt.bfloat16
F16 = mybir.dt.float16

# The x activations, QKV projection weights, and output partials are shipped
# in fp16: DMA bandwidth is the measured bottleneck (~150 GB/s/core), and
# fp16's 10-bit mantissa keeps the end-to-end error within ~2x of the
# all-fp32r pipeline. On-chip tensors stay fp32r.
XDT = F16
ODT = F16
AF = mybir.ActivationFunctionType
ALU = mybir.AluOpType

MASK_NEG = -1.0e30


def _build_bass(bench_iters=None):
    nc = bacc.Bacc("TRN2", target_bir_lowering=False, debug=False,
                   num_devices=NCORES)

    xqT = nc.dram_tensor("xqT", [D, N], XDT, kind="ExternalInput").ap()
    xkT = nc.dram_tensor("xkT", [D, N], XDT, kind="ExternalInput").ap()
    wqT = nc.dram_tensor("wqT", [D, M], XDT, kind="ExternalInput").ap()
    wkT = nc.dram_tensor("wkT", [D, M], XDT, kind="ExternalInput").ap()
    wvT = nc.dram_tensor("wvT", [D, M], XDT, kind="ExternalInput").ap()
    woT = nc.dram_tensor("woT", [M, D], F32R, kind="ExternalInput").ap()
    msk = nc.dram_tensor("msk", [128, 4 * IC], F32, kind="ExternalInput").ap()
    out = nc.dram_tensor("out", [N, D], ODT, kind="ExternalOutput").ap()

    with tile.TileContext(nc) as tc:
        with (
            tc.tile_pool(name="wts", bufs=1) as wpool,
            tc.tile_pool(name="xs", bufs=3) as xpool,
            tc.tile_pool(name="acts", bufs=2) as apool,
            tc.tile_pool(name="ps", bufs=4) as ppool,
            tc.tile_pool(name="qkv", bufs=2, space="PSUM") as qkvps,
            tc.tile_pool(name="sc", bufs=2, space="PSUM") as scps,
            tc.tile_pool(name="pv", bufs=2, space="PSUM") as pvps,
            tc.tile_pool(name="op", bufs=2, space="PSUM") as opps,
        ):
            # --- constants ---
            wq_sb = wpool.tile([128, 8 * 128], XDT, tag="wq")
            wk_sb = wpool.tile([128, 8 * 128], XDT, tag="wk")
            wv_sb = wpool.tile([128, 8 * 128], XDT, tag="wv")
            wo_sb = wpool.tile([128, D], F32R, tag="wo")
            msk_sb = wpool.tile([128, 4 * IC], F32, tag="msk")
            idn = wpool.tile([128, 128], F32, tag="idn")
            nc.sync.dma_start(wq_sb.rearrange("p (c m) -> p c m", m=128),
                              wqT.rearrange("(c p) m -> p c m", p=128))
            nc.sync.dma_start(wk_sb.rearrange("p (c m) -> p c m", m=128),
                              wkT.rearrange("(c p) m -> p c m", p=128))
            nc.sync.dma_start(wv_sb.rearrange("p (c m) -> p c m", m=128),
                              wvT.rearrange("(c p) m -> p c m", p=128))
            nc.sync.dma_start(wo_sb[:], woT[:, :])
            nc.sync.dma_start(msk_sb[:], msk[:, :])
            make_identity(nc, idn[:])

            from contextlib import nullcontext
            loop_cm = (tc.For_i(0, bench_iters, 1)
                       if bench_iters else nullcontext())
            with loop_cm:
                _emit_body(nc, tc, locals())
    nc.finalize()
    return nc


def _emit_body(nc, tc, env):
    (xqT, xkT, msk_sb, out, wq_sb, wk_sb, wv_sb, wo_sb, idn,
     xpool, apool, ppool, qkvps, scps, pvps, opps) = (
        env["xqT"], env["xkT"], env["msk_sb"], env["out"], env["wq_sb"],
        env["wk_sb"], env["wv_sb"], env["wo_sb"], env["idn"], env["xpool"],
        env["apool"], env["ppool"], env["qkvps"], env["scps"], env["pvps"],
        env["opps"])
    if True:
            for b in range(B):
                # per-batch activation tiles (bufs=2 double-buffers batches)
                qT = apool.tile([128, S], F32R, tag="qT")
                kT = apool.tile([128, S], F32R, tag="kT")
                # v_comb: per (j-tile, head) slot [128, 128]: cols 0-63 v,
                # cols 64-127 ones (for the fused row-sum)
                vc = apool.tile([128, NJT * HPC * 128], F32R, tag="vc")
                aT = apool.tile([128, S], F32R, tag="aT")

                ones_ap = vc.bitcast(F32).rearrange(
                    "p (s two c) -> p s two c", two=2, c=64)[:, :, 1, :]
                nc.gpsimd.memset(ones_ap, 1.0)

                # Interleaved per-i-chunk pipeline: QKV(icb) -> causal
                # attention(icb) -> partial out-projection(icb). Later
                # i-chunks' QKV DMA overlaps earlier chunks' attention.
                for icb in range(NICB):
                    i0 = b * S + icb * IC
                    # ---- QKV projections for this i-chunk ----
                    # One big DMA per source: all 8 d-chunks [128, 512]
                    xq_t = xpool.tile([128, 8, IC], XDT, tag="xa")
                    xk_t = xpool.tile([128, 8, IC], XDT, tag="xa")
                    for g in range(4):
                        nc.sync.dma_start(
                            xq_t[:, ds(2 * g, 2), :],
                            xqT[ds(2 * g * 128, 256), ds(i0, IC)].rearrange(
                                "(c p) i -> p c i", p=128))
                        nc.sync.dma_start(
                            xk_t[:, ds(2 * g, 2), :],
                            xkT[ds(2 * g * 128, 256), ds(i0, IC)].rearrange(
                                "(c p) i -> p c i", p=128))
                    for which, w_sb, x_t in (("q", wq_sb, xq_t),
                                             ("k", wk_sb, xk_t),
                                             ("v", wv_sb, xk_t)):
                        ps = qkvps.tile([128, IC], F32, tag="qkv",
                                        name=f"ps_{b}_{icb}_{which}")
                        for dc in range(8):
                            nc.tensor.matmul(ps[:], w_sb[:, ds(dc * 128, 128)],
                                             x_t[:, dc, :],
                                             start=(dc == 0), stop=(dc == 7))
                        if which == "q":
                            nc.vector.tensor_copy(qT[:, ds(icb * IC, IC)], ps[:])
                        elif which == "k":
                            nc.vector.tensor_copy(kT[:, ds(icb * IC, IC)], ps[:])
                        else:
                            # v -> natural [j, m] layout via PE transpose
                            vt_t = xpool.tile([128, IC], F32, tag="vt")
                            nc.vector.tensor_copy(vt_t[:], ps[:])
                            tp_ps = opps.tile([128, IC], F32, tag="op")
                            for t in range(4):
                                nc.tensor.transpose(tp_ps[:, ds(t * 128, 128)],
                                                    vt_t[:, ds(t * 128, 128)],
                                                    idn[:])
                            for t in range(4):
                                jt = icb * 4 + t
                                for h in range(HPC):
                                    nc.vector.tensor_copy(
                                        vc[:, ds((jt * HPC + h) * 128, 64)],
                                        tp_ps[:, ds(t * 128 + h * 64, 64)])

                    # ---- causal attention for this i-chunk ----
                    njt = 4 * icb + 4
                    pv_tiles = [pvps.tile([128, IC], F32, tag="pv",
                                          name=f"pv_{b}_{icb}_{h}")
                                for h in range(HPC)]
                    for jt in range(njt):
                        # both heads' K=64 score matmuls back-to-back: they
                        # target disjoint PE row-groups (partitions 0-63 /
                        # 64-127) and run concurrently in the array
                        s_tiles, p_ts = [], []
                        for h in range(HPC):
                            s_ps = scps.tile([128, IC], F32, tag="sc",
                                             name=f"s_{b}_{icb}_{jt}_{h}")
                            nc.tensor.matmul(
                                s_ps[:],
                                kT[ds(h * 64, 64), ds(jt * 128, 128)],
                                qT[ds(h * 64, 64), ds(icb * IC, IC)],
                                start=True, stop=True)
                            s_tiles.append(s_ps)
                        for h in range(HPC):
                            p_t = ppool.tile([128, IC], F32R, tag="p",
                                             name=f"p_{b}_{icb}_{jt}_{h}")
                            nc.scalar.activation(p_t[:], s_tiles[h][:], AF.Exp)
                            if jt >= 4 * icb:  # diagonal block: causal mask
                                rr = jt - 4 * icb
                                nc.vector.tensor_tensor(
                                    p_t[:], p_t[:],
                                    msk_sb[:, ds(rr * IC, IC)], ALU.mult)
                            p_ts.append(p_t)
                        for h in range(HPC):
                            nc.tensor.matmul(
                                pv_tiles[h][:],
                                vc[:, ds((jt * HPC + h) * 128, 128)],
                                p_ts[h][:],
                                start=(jt == 0), stop=(jt == njt - 1))
                    for h in range(HPC):
                        rc_t = ppool.tile([64, IC], F32, tag="rc")
                        nc.vector.reciprocal(rc_t[:], pv_tiles[h][ds(64, 64), :])
                        nc.vector.tensor_tensor(
                            aT[ds(h * 64, 64), ds(icb * IC, IC)],
                            pv_tiles[h][ds(0, 64), :],
                            rc_t[:], ALU.mult)

                    # ---- partial out-projection for this i-chunk ----
                    for i128 in range(4):
                        ii = icb * 4 + i128
                        for dn in range(D // IC):
                            o_ps = opps.tile([128, IC], F32, tag="op")
                            nc.tensor.matmul(o_ps[:],
                                             aT[:, ds(ii * 128, 128)],
                                             wo_sb[:, ds(dn * IC, IC)],
                                             start=True, stop=True)
                            o_sb = ppool.tile([128, IC], ODT, tag="osb")
                            if dn == 0:
                                nc.vector.tensor_copy(o_sb[:], o_ps[:])
                            else:
                                nc.scalar.copy(o_sb[:], o_ps[:])
                            nc.sync.dma_start(
                                out[ds(b * S + ii * 128, 128), ds(dn * IC, IC)],
                                o_sb[:])


_STATE = {}


def _get_runner(bench_iters=None):
    """Build the Bass module and a cached jitted SPMD executor (compile once)."""
    global _STATE
    if bench_iters in _STATE:
        return _STATE[bench_iters]

    import jax
    from jax.sharding import Mesh, PartitionSpec
    from jax.experimental.shard_map import shard_map
    from concourse import bass2jax

    bass2jax.install_neuronx_cc_hook()
    nc = _build_bass(bench_iters)

    partition_name = (nc.partition_id_tensor.name
                      if nc.partition_id_tensor else None)
    in_names, out_names, out_avals, zero_shapes = [], [], [], []
    for alloc in nc.m.functions[0].allocations:
        if not isinstance(alloc, mybir.MemoryLocationSet):
            continue
        name = alloc.memorylocations[0].name
        if alloc.kind == "ExternalInput":
            if name != partition_name:
                in_names.append(name)
        elif alloc.kind == "ExternalOutput":
            shape = tuple(alloc.tensor_shape)
            dtype = mybir.dt.np(alloc.dtype)
            out_names.append(name)
            out_avals.append(jax.core.ShapedArray(shape, dtype))
            zero_shapes.append((shape, dtype))
    n_params = len(in_names)
    n_outs = len(out_avals)
    all_in_names = list(in_names) + list(out_names)
    if partition_name is not None:
        all_in_names.append(partition_name)

    def _body(*args):
        operands = list(args)
        if partition_name is not None:
            operands.append(bass2jax.partition_id_tensor())
        outs = bass2jax._bass_exec_p.bind(
            *operands,
            out_avals=tuple(out_avals),
            in_names=tuple(all_in_names),
            out_names=tuple(out_names),
            lowering_input_output_aliases=(),
            sim_require_finite=True,
            sim_require_nnan=True,
            nc=nc,
        )
        return tuple(outs)

    devices = jax.devices()[:NCORES]
    mesh = Mesh(np.asarray(devices), ("core",))
    in_specs = (PartitionSpec("core"),) * (n_params + n_outs)
    out_specs = (PartitionSpec("core"),) * n_outs
    donate = tuple(range(n_params, n_params + n_outs))
    sharded = jax.jit(
        shard_map(_body, mesh=mesh, in_specs=in_specs, out_specs=out_specs,
                  check_rep=False),
        donate_argnums=donate, keep_unused=True)

    def run(in_maps):
        concat_in = [
            np.concatenate([np.asarray(in_maps[c][k]) for c in range(NCORES)],
                           axis=0)
            for k in in_names
        ]
        concat_zeros = [np.zeros((NCORES * s[0], *s[1:]), dt)
                        for s, dt in zero_shapes]
        out_arrs = sharded(*concat_in, *concat_zeros)
        return [
            {k: np.asarray(out_arrs[i]).reshape(NCORES, *out_avals[i].shape)[c]
             for i, k in enumerate(out_names)}
            for c in range(NCORES)
        ]

    _STATE[bench_iters] = run
    return run


def _make_mask():
    """msk[jj, rr*512 + ii] = 1 if ii >= jj + 128*rr else 0 (multiplicative)."""
    jj = np.arange(128)[:, None]
    ii = np.arange(IC)[None, :]
    tiles = [np.where(ii >= jj + 128 * rr, 1.0, 0.0).astype(np.float32)
             for rr in range(4)]
    return np.concatenate(tiles, axis=1)


def prepare_in_maps(inputs_q, inputs_kv, Wq, Wk, Wv, Wo):
    xdt = np.float16
    xq = np.ascontiguousarray(
        np.asarray(inputs_q, np.float32).reshape(N, D).T.astype(xdt))
    xk = np.ascontiguousarray(
        np.asarray(inputs_kv, np.float32).reshape(N, D).T.astype(xdt))
    Wq = np.asarray(Wq, np.float32)
    Wk = np.asarray(Wk, np.float32)
    Wv = np.asarray(Wv, np.float32)
    Wo = np.asarray(Wo, np.float32)
    msk = _make_mask()
    scale = 1.0 / np.sqrt(np.float32(Dh))
    in_maps = []
    for c in range(NCORES):
        sl = slice(c * M, (c + 1) * M)
        in_maps.append({
            "xqT": xq,
            "xkT": xk,
            "wqT": np.ascontiguousarray((Wq[sl, :] * scale).T.astype(xdt)),
            "wkT": np.ascontiguousarray(Wk[sl, :].T.astype(xdt)),
            "wvT": np.ascontiguousarray(Wv[sl, :].T.astype(xdt)),
            "woT": np.ascontiguousarray(Wo[:, sl].T),
            "msk": msk,
        })
    return in_maps


def kernel(inputs_q, inputs_kv, mask, Wq, Wk, Wv, Wo):
    run = _get_runner()
    in_maps = prepare_in_maps(inputs_q, inputs_kv, Wq, Wk, Wv, Wo)
    results = run(in_maps)
    acc = results[0]["out"].astype(np.float32)
    for c in range(1, NCORES):
        acc = acc + results[c]["out"]
    return acc.reshape(B, S, D)


# revision 22
# speedup vs baseline: 1.6439x; 1.2851x over previous
"""Multi-head causal attention (B=2, S=2048, H=16, Dh=64) on 8 TRN2 NeuronCores.

Sharding: tensor-parallel over heads — core c owns heads [2c, 2c+1] (a
128-wide feature block) for both batches. Each core computes its heads'
QKV projections, causal attention, and a partial output projection
(attn_c @ Wo[:, 128c:128c+128].T); the host sums the 8 partials
(the all-reduce) and reshapes.

On-chip layout keeps the feature dim on SBUF partitions throughout
("T layout"), so scores are computed directly as S.T[j, i] and the
post-softmax matrix needs no transpose before the PV matmul. The softmax
row-sum is obtained for free by augmenting V with 64 columns of ones
inside the PV matmul; exp is unnormalized (scores are O(1), no max
subtraction needed) and the division happens once per output tile.
"""

import numpy as np

import concourse.bass as bass
import concourse.mybir as mybir
import concourse.tile as tile
from concourse import bacc
from concourse.bass import ds
from concourse.masks import make_identity

B, S, H, Dh = 2, 2048, 16, 64
D = H * Dh            # 1024
NCORES = 8
HPC = H // NCORES     # heads per core = 2
M = HPC * Dh          # per-core feature block = 128
N = B * S             # 4096 token rows
IC = 512              # i-chunk (matmul moving free dim)
NICB = S // IC        # 4 i-chunks per batch
NJT = S // 128        # 16 j-tiles per batch

F32 = mybir.dt.float32
F32R = mybir.dt.float32r
BF16 = mybir.dt.bfloat16
F16 = mybir.dt.float16

# The x activations and QKV projection weights are shipped and multiplied in
# bf16: DMA bandwidth is the measured bottleneck (~150 GB/s/core) and the PE
# only runs bf16/fp8 at full rate (fp16 matmuls measured ~4x slower, fp32r
# needs 4-byte streams). Output partials ship as fp16 (better mantissa than
# bf16 at the same DMA cost); on-chip attention tensors stay fp32r.
XDT = BF16
ODT = F16
AF = mybir.ActivationFunctionType
ALU = mybir.AluOpType

MASK_NEG = -1.0e30


def _build_bass(bench_iters=None):
    nc = bacc.Bacc("TRN2", target_bir_lowering=False, debug=False,
                   num_devices=NCORES)

    xqT = nc.dram_tensor("xqT", [D, N], XDT, kind="ExternalInput").ap()
    xkT = nc.dram_tensor("xkT", [D, N], XDT, kind="ExternalInput").ap()
    wqT = nc.dram_tensor("wqT", [D, M], XDT, kind="ExternalInput").ap()
    wkT = nc.dram_tensor("wkT", [D, M], XDT, kind="ExternalInput").ap()
    wvT = nc.dram_tensor("wvT", [D, M], XDT, kind="ExternalInput").ap()
    woT = nc.dram_tensor("woT", [M, D], F32R, kind="ExternalInput").ap()
    msk = nc.dram_tensor("msk", [128, 4 * IC], F32, kind="ExternalInput").ap()
    out = nc.dram_tensor("out", [N, D], ODT, kind="ExternalOutput").ap()

    with tile.TileContext(nc) as tc:
        with (
            tc.tile_pool(name="wts", bufs=1) as wpool,
            tc.tile_pool(name="xs", bufs=3) as xpool,
            tc.tile_pool(name="acts", bufs=2) as apool,
            tc.tile_pool(name="ps", bufs=4) as ppool,
            tc.tile_pool(name="qkv", bufs=2, space="PSUM") as qkvps,
            tc.tile_pool(name="sc", bufs=2, space="PSUM") as scps,
            tc.tile_pool(name="pv", bufs=2, space="PSUM") as pvps,
            tc.tile_pool(name="op", bufs=2, space="PSUM") as opps,
        ):
            # --- constants ---
            wq_sb = wpool.tile([128, 8 * 128], XDT, tag="wq")
            wk_sb = wpool.tile([128, 8 * 128], XDT, tag="wk")
            wv_sb = wpool.tile([128, 8 * 128], XDT, tag="wv")
            wo_sb = wpool.tile([128, D], F32R, tag="wo")
            msk_sb = wpool.tile([128, 4 * IC], F32, tag="msk")
            idn = wpool.tile([128, 128], F32, tag="idn")
            nc.sync.dma_start(wq_sb.rearrange("p (c m) -> p c m", m=128),
                              wqT.rearrange("(c p) m -> p c m", p=128))
            nc.sync.dma_start(wk_sb.rearrange("p (c m) -> p c m", m=128),
                              wkT.rearrange("(c p) m -> p c m", p=128))
            nc.sync.dma_start(wv_sb.rearrange("p (c m) -> p c m", m=128),
                              wvT.rearrange("(c p) m -> p c m", p=128))
            nc.sync.dma_start(wo_sb[:], woT[:, :])
            nc.sync.dma_start(msk_sb[:], msk[:, :])
            make_identity(nc, idn[:])

            from contextlib import nullcontext
            loop_cm = (tc.For_i(0, bench_iters, 1)
                       if bench_iters else nullcontext())
            with loop_cm:
                _emit_body(nc, tc, locals())
    nc.finalize()
    return nc


def _emit_body(nc, tc, env):
    (xqT, xkT, msk_sb, out, wq_sb, wk_sb, wv_sb, wo_sb, idn,
     xpool, apool, ppool, qkvps, scps, pvps, opps) = (
        env["xqT"], env["xkT"], env["msk_sb"], env["out"], env["wq_sb"],
        env["wk_sb"], env["wv_sb"], env["wo_sb"], env["idn"], env["xpool"],
        env["apool"], env["ppool"], env["qkvps"], env["scps"], env["pvps"],
        env["opps"])
    if True:
            for b in range(B):
                # per-batch activation tiles (bufs=2 double-buffers batches)
                qT = apool.tile([128, S], F32R, tag="qT")
                kT = apool.tile([128, S], F32R, tag="kT")
                # v_comb: per (j-tile, head) slot [128, 128]: cols 0-63 v,
                # cols 64-127 ones (for the fused row-sum)
                vc = apool.tile([128, NJT * HPC * 128], F32R, tag="vc")
                aT = apool.tile([128, S], F32R, tag="aT")

                ones_ap = vc.bitcast(F32).rearrange(
                    "p (s two c) -> p s two c", two=2, c=64)[:, :, 1, :]
                nc.gpsimd.memset(ones_ap, 1.0)

                # Interleaved per-i-chunk pipeline: QKV(icb) -> causal
                # attention(icb) -> partial out-projection(icb). Later
                # i-chunks' QKV DMA overlaps earlier chunks' attention.
                for icb in range(NICB):
                    i0 = b * S + icb * IC
                    # ---- QKV projections for this i-chunk ----
                    # One big DMA per source: all 8 d-chunks [128, 512]
                    xq_t = xpool.tile([128, 8, IC], XDT, tag="xa")
                    xk_t = xpool.tile([128, 8, IC], XDT, tag="xa")
                    for g in range(4):
                        nc.sync.dma_start(
                            xq_t[:, ds(2 * g, 2), :],
                            xqT[ds(2 * g * 128, 256), ds(i0, IC)].rearrange(
                                "(c p) i -> p c i", p=128))
                        nc.sync.dma_start(
                            xk_t[:, ds(2 * g, 2), :],
                            xkT[ds(2 * g * 128, 256), ds(i0, IC)].rearrange(
                                "(c p) i -> p c i", p=128))
                    for which, w_sb, x_t in (("q", wq_sb, xq_t),
                                             ("k", wk_sb, xk_t),
                                             ("v", wv_sb, xk_t)):
                        ps = qkvps.tile([128, IC], F32, tag="qkv",
                                        name=f"ps_{b}_{icb}_{which}")
                        for dc in range(8):
                            nc.tensor.matmul(ps[:], w_sb[:, ds(dc * 128, 128)],
                                             x_t[:, dc, :],
                                             start=(dc == 0), stop=(dc == 7))
                        if which == "q":
                            nc.vector.tensor_copy(qT[:, ds(icb * IC, IC)], ps[:])
                        elif which == "k":
                            nc.vector.tensor_copy(kT[:, ds(icb * IC, IC)], ps[:])
                        else:
                            # v -> natural [j, m] layout via PE transpose
                            vt_t = xpool.tile([128, IC], F32, tag="vt")
                            nc.vector.tensor_copy(vt_t[:], ps[:])
                            tp_ps = opps.tile([128, IC], F32, tag="op")
                            for t in range(4):
                                nc.tensor.transpose(tp_ps[:, ds(t * 128, 128)],
                                                    vt_t[:, ds(t * 128, 128)],
                                                    idn[:])
                            for t in range(4):
                                jt = icb * 4 + t
                                for h in range(HPC):
                                    nc.vector.tensor_copy(
                                        vc[:, ds((jt * HPC + h) * 128, 64)],
                                        tp_ps[:, ds(t * 128 + h * 64, 64)])

                    # ---- causal attention for this i-chunk ----
                    njt = 4 * icb + 4
                    pv_tiles = [pvps.tile([128, IC], F32, tag="pv",
                                          name=f"pv_{b}_{icb}_{h}")
                                for h in range(HPC)]
                    for jt in range(njt):
                        # both heads' K=64 score matmuls back-to-back: they
                        # target disjoint PE row-groups (partitions 0-63 /
                        # 64-127) and run concurrently in the array
                        s_tiles, p_ts = [], []
                        for h in range(HPC):
                            s_ps = scps.tile([128, IC], F32, tag="sc",
                                             name=f"s_{b}_{icb}_{jt}_{h}")
                            nc.tensor.matmul(
                                s_ps[:],
                                kT[ds(h * 64, 64), ds(jt * 128, 128)],
                                qT[ds(h * 64, 64), ds(icb * IC, IC)],
                                start=True, stop=True)
                            s_tiles.append(s_ps)
                        for h in range(HPC):
                            p_t = ppool.tile([128, IC], F32R, tag="p",
                                             name=f"p_{b}_{icb}_{jt}_{h}")
                            nc.scalar.activation(p_t[:], s_tiles[h][:], AF.Exp)
                            if jt >= 4 * icb:  # diagonal block: causal mask
                                rr = jt - 4 * icb
                                nc.vector.tensor_tensor(
                                    p_t[:], p_t[:],
                                    msk_sb[:, ds(rr * IC, IC)], ALU.mult)
                            p_ts.append(p_t)
                        for h in range(HPC):
                            nc.tensor.matmul(
                                pv_tiles[h][:],
                                vc[:, ds((jt * HPC + h) * 128, 128)],
                                p_ts[h][:],
                                start=(jt == 0), stop=(jt == njt - 1))
                    for h in range(HPC):
                        rc_t = ppool.tile([64, IC], F32, tag="rc")
                        nc.vector.reciprocal(rc_t[:], pv_tiles[h][ds(64, 64), :])
                        nc.vector.tensor_tensor(
                            aT[ds(h * 64, 64), ds(icb * IC, IC)],
                            pv_tiles[h][ds(0, 64), :],
                            rc_t[:], ALU.mult)

                    # ---- partial out-projection for this i-chunk ----
                    for i128 in range(4):
                        ii = icb * 4 + i128
                        for dn in range(D // IC):
                            o_ps = opps.tile([128, IC], F32, tag="op")
                            nc.tensor.matmul(o_ps[:],
                                             aT[:, ds(ii * 128, 128)],
                                             wo_sb[:, ds(dn * IC, IC)],
                                             start=True, stop=True)
                            o_sb = ppool.tile([128, IC], ODT, tag="osb")
                            if dn == 0:
                                nc.vector.tensor_copy(o_sb[:], o_ps[:])
                            else:
                                nc.scalar.copy(o_sb[:], o_ps[:])
                            nc.sync.dma_start(
                                out[ds(b * S + ii * 128, 128), ds(dn * IC, IC)],
                                o_sb[:])


_STATE = {}


def _get_runner(bench_iters=None):
    """Build the Bass module and a cached jitted SPMD executor (compile once)."""
    global _STATE
    if bench_iters in _STATE:
        return _STATE[bench_iters]

    import jax
    from jax.sharding import Mesh, PartitionSpec
    from jax.experimental.shard_map import shard_map
    from concourse import bass2jax

    bass2jax.install_neuronx_cc_hook()
    nc = _build_bass(bench_iters)

    partition_name = (nc.partition_id_tensor.name
                      if nc.partition_id_tensor else None)
    in_names, out_names, out_avals, zero_shapes = [], [], [], []
    for alloc in nc.m.functions[0].allocations:
        if not isinstance(alloc, mybir.MemoryLocationSet):
            continue
        name = alloc.memorylocations[0].name
        if alloc.kind == "ExternalInput":
            if name != partition_name:
                in_names.append(name)
        elif alloc.kind == "ExternalOutput":
            shape = tuple(alloc.tensor_shape)
            dtype = mybir.dt.np(alloc.dtype)
            out_names.append(name)
            out_avals.append(jax.core.ShapedArray(shape, dtype))
            zero_shapes.append((shape, dtype))
    n_params = len(in_names)
    n_outs = len(out_avals)
    all_in_names = list(in_names) + list(out_names)
    if partition_name is not None:
        all_in_names.append(partition_name)

    def _body(*args):
        operands = list(args)
        if partition_name is not None:
            operands.append(bass2jax.partition_id_tensor())
        outs = bass2jax._bass_exec_p.bind(
            *operands,
            out_avals=tuple(out_avals),
            in_names=tuple(all_in_names),
            out_names=tuple(out_names),
            lowering_input_output_aliases=(),
            sim_require_finite=True,
            sim_require_nnan=True,
            nc=nc,
        )
        return tuple(outs)

    devices = jax.devices()[:NCORES]
    mesh = Mesh(np.asarray(devices), ("core",))
    in_specs = (PartitionSpec("core"),) * (n_params + n_outs)
    out_specs = (PartitionSpec("core"),) * n_outs
    donate = tuple(range(n_params, n_params + n_outs))
    sharded = jax.jit(
        shard_map(_body, mesh=mesh, in_specs=in_specs, out_specs=out_specs,
                  check_rep=False),
        donate_argnums=donate, keep_unused=True)

    def run(in_maps):
        concat_in = [
            np.concatenate([np.asarray(in_maps[c][k]) for c in range(NCORES)],
                           axis=0)
            for k in in_names
        ]
        concat_zeros = [np.zeros((NCORES * s[0], *s[1:]), dt)
                        for s, dt in zero_shapes]
        out_arrs = sharded(*concat_in, *concat_zeros)
        return [
            {k: np.asarray(out_arrs[i]).reshape(NCORES, *out_avals[i].shape)[c]
             for i, k in enumerate(out_names)}
            for c in range(NCORES)
        ]

    _STATE[bench_iters] = run
    return run


def _make_mask():
    """msk[jj, rr*512 + ii] = 1 if ii >= jj + 128*rr else 0 (multiplicative)."""
    jj = np.arange(128)[:, None]
    ii = np.arange(IC)[None, :]
    tiles = [np.where(ii >= jj + 128 * rr, 1.0, 0.0).astype(np.float32)
             for rr in range(4)]
    return np.concatenate(tiles, axis=1)


def prepare_in_maps(inputs_q, inputs_kv, Wq, Wk, Wv, Wo):
    import ml_dtypes
    xdt = ml_dtypes.bfloat16
    xq = np.ascontiguousarray(
        np.asarray(inputs_q, np.float32).reshape(N, D).T.astype(xdt))
    xk = np.ascontiguousarray(
        np.asarray(inputs_kv, np.float32).reshape(N, D).T.astype(xdt))
    Wq = np.asarray(Wq, np.float32)
    Wk = np.asarray(Wk, np.float32)
    Wv = np.asarray(Wv, np.float32)
    Wo = np.asarray(Wo, np.float32)
    msk = _make_mask()
    scale = 1.0 / np.sqrt(np.float32(Dh))
    in_maps = []
    for c in range(NCORES):
        sl = slice(c * M, (c + 1) * M)
        in_maps.append({
            "xqT": xq,
            "xkT": xk,
            "wqT": np.ascontiguousarray((Wq[sl, :] * scale).T.astype(xdt)),
            "wkT": np.ascontiguousarray(Wk[sl, :].T.astype(xdt)),
            "wvT": np.ascontiguousarray(Wv[sl, :].T.astype(xdt)),
            "woT": np.ascontiguousarray(Wo[:, sl].T),
            "msk": msk,
        })
    return in_maps


def _run_fallback(in_maps):
    """Slow-but-sure path: the stock SPMD runner (fresh compile per call)."""
    from concourse.bass_utils import run_bass_kernel_spmd
    nc = _build_bass()
    res = run_bass_kernel_spmd(nc, in_maps, core_ids=list(range(NCORES)))
    return res.results


def kernel(inputs_q, inputs_kv, mask, Wq, Wk, Wv, Wo):
    in_maps = prepare_in_maps(inputs_q, inputs_kv, Wq, Wk, Wv, Wo)
    try:
        results = _get_runner()(in_maps)
    except Exception:
        results = _run_fallback(in_maps)
    acc = results[0]["out"].astype(np.float32)
    for c in range(1, NCORES):
        acc = acc + results[c]["out"]
    return acc.reshape(B, S, D)
